# revision 10
# baseline (speedup 1.0000x reference)
"""Area-attention (AAttn) Trainium2 kernel: 8-core SPMD, no collectives.

Shards the B*AREA=16 (batch, area-stripe) pairs 2-per-core; each stripe is
16 rows x 64 cols of the 64x64 image. The depthwise-conv halo (3 rows) is
recomputed locally from a halo'd x slice, so cores never communicate.

Layout: channels host-permuted so qkv rows are [q(h-major)|k|v]; attention
runs in S^T orientation (m on partitions) so softmax needs no transpose:
exp via ACT, Z via a ones-column in the PV matmul (M=33), normalization via
a select-matrix broadcast matmul. Depthwise 7x7 runs as PE diag-matmul taps
plus DVE scalar_tensor_tensor taps on a zero-padded v tile.
"""

import sys

sys.path.insert(0, "/opt/trn_rl_repo")

import ml_dtypes
import numpy as np

import concourse.bass as bass
from concourse import bacc
import concourse.mybir as mybir
import concourse.tile as tile
from concourse.bass_utils import run_bass_kernel_spmd

B, C, H, W = 4, 256, 64, 64
HEADS, AREA, HD = 8, 4, 32
N = H * W
NA = N // AREA          # 1024 pixels per stripe
BA = B * AREA           # 16 stripes
NCORES = 8
SPC = BA // NCORES      # 2 stripes per core
ROWS = H // AREA        # 16
HALO = 3
PROWS = ROWS + 2 * HALO  # 22
PCOLS = W + 2 * HALO     # 70
PFREE = PROWS * PCOLS    # 1540
XFREE = PROWS * W        # 1408
XI0 = HALO * W           # interior pixel offset in x tile
SCALE = HD ** -0.5
EPS = 1e-5

F32 = mybir.dt.float32
BF16 = mybir.dt.bfloat16
BF16NP = ml_dtypes.bfloat16

# tap t = 7*dy + dx ; PE taps accumulate via diag-matmul in PSUM,
# DVE taps via scalar_tensor_tensor FMA on shifted window APs.
N_PE_TAPS = 36
PE_TAPS = list(range(N_PE_TAPS))
DVE_TAPS = list(range(N_PE_TAPS, 49))
PE_SEG = 33

_CACHE = {}
DEBUG = False
DW_INTERLEAVE = True


def _fold_bn(g, b, m, v):
    s = np.asarray(g) / np.sqrt(np.asarray(v) + EPS)
    return s, np.asarray(b) - np.asarray(m) * s


def _host_prep(inp):
    x = np.asarray(inp["x"], np.float32)
    qkv_w = np.asarray(inp["qkv_w"], np.float32)[:, :, 0, 0]
    proj_w = np.asarray(inp["proj_w"], np.float32)[:, :, 0, 0]
    pe_w = np.asarray(inp["pe_w"], np.float32)[:, 0]

    qs, qb = _fold_bn(inp["qkv_g"], inp["qkv_b"], inp["qkv_m"], inp["qkv_v"])
    Wqkv = qkv_w * qs[:, None]
    ps_, pb = _fold_bn(inp["pe_g"], inp["pe_b"], inp["pe_m"], inp["pe_v"])
    Wpe = pe_w * ps_[:, None, None]
    rs, rb = _fold_bn(inp["proj_g"], inp["proj_b"], inp["proj_m"], inp["proj_v"])
    Wp = proj_w * rs[:, None]

    perm = np.empty(3 * C, np.int64)
    for h in range(HEADS):
        for d in range(HD):
            perm[h * HD + d] = 96 * h + d
            perm[C + h * HD + d] = 96 * h + 32 + d
            perm[2 * C + h * HD + d] = 96 * h + 64 + d
    Wqkv_p = Wqkv[perm]  # noqa
    bqkv_p = qb[perm].astype(np.float32)
    wqkvT = np.ascontiguousarray(Wqkv_p.T)          # (256, 768)
    Wv = Wqkv_p[2 * C:]
    bv = bqkv_p[2 * C:]

    # proj bias absorbs: proj's own BN shift, pe's BN shift, and the
    # attention-v bias (out = v0@E/Z + bv since softmax rows sum to 1)
    bp = (rb + Wp @ pb + Wp @ bv).astype(np.float32)

    wvT_aug = np.zeros((C, HEADS * 33), np.float32)
    for h in range(HEADS):
        wvT_aug[:, 33 * h:33 * h + 32] = Wv[32 * h:32 * h + 32].T
    onehot = np.zeros((1, HEADS * 33), np.float32)
    onehot[0, 32::33] = 1.0

    selk = np.zeros((128, 2 * 97), np.float32)
    selk[:, 0:33] = 1.0
    selk[:, 97 + 64:97 + 97] = 1.0

    dwdiag = np.zeros((max(1, N_PE_TAPS) * 2, 128, 128), np.float32)
    for i, t in enumerate(PE_TAPS):
        dy, dx = t // 7, t % 7
        for ch in range(2):
            np.fill_diagonal(dwdiag[2 * i + ch], Wpe[128 * ch:128 * ch + 128, dy, dx])
    nd = max(1, len(DVE_TAPS))
    dwvec = np.zeros((2, 128, nd), np.float32)
    for i, t in enumerate(DVE_TAPS):
        dy, dx = t // 7, t % 7
        for ch in range(2):
            dwvec[ch, :, i] = Wpe[128 * ch:128 * ch + 128, dy, dx]

    bias6 = np.zeros((128, 6), np.float32)
    for j in range(6):
        bias6[:, j] = bqkv_p[128 * j:128 * j + 128]
    brow = np.zeros((1, 512), np.float32)
    brow[0, :256] = bv
    brow[0, 256:] = bp
    ones = np.ones((1, 1024), np.float32)

    x2 = np.zeros((NCORES, SPC, 2, 128, PROWS, W), np.float32)
    rowmask = np.zeros((NCORES, SPC, 1, 2 * HALO * W), np.float32)
    for core in range(NCORES):
        for s in range(SPC):
            gs = core * SPC + s
            b_, a_ = gs // AREA, gs % AREA
            r0 = a_ * ROWS - HALO
            for r in range(PROWS):
                rr = r0 + r
                if 0 <= rr < H:
                    x2[core, s, :, :, r] = x[b_, :, rr].reshape(2, 128, W)
            rowmask[core, s, 0, :HALO * W] = 1.0 if a_ > 0 else 0.0
            rowmask[core, s, 0, HALO * W:] = 1.0 if a_ < AREA - 1 else 0.0

    def bf(a):
        return np.ascontiguousarray(a).astype(BF16NP)

    shared = {
        "wqkvT": bf(wqkvT.reshape(2, 128, 768)),
        "wvT": bf(wvT_aug.reshape(2, 128, 264)),
        "wpT": bf(np.ascontiguousarray(Wp.T).reshape(2, 128, 256)),
        "dwdiag": bf(dwdiag.transpose(1, 0, 2).reshape(128, N_PE_TAPS * 2 * 128)) if N_PE_TAPS else np.zeros((128, 1), BF16NP),
        "dwvec": np.ascontiguousarray(dwvec),
        "bias6": bias6, "brow": brow, "ones": ones,
        "onehot": onehot, "selk": selk,
    }
    in_maps = []
    for core in range(NCORES):
        m = dict(shared)
        m["x2"] = bf(x2[core].reshape(SPC, 2, 128, XFREE))
        m["rowmask"] = np.ascontiguousarray(rowmask[core])
        in_maps.append(m)
    return in_maps


def _build():
    nc = bacc.Bacc("TRN2", target_bir_lowering=False, debug=False)

    d_x2 = nc.dram_tensor("x2", [SPC, 2, 128, XFREE], BF16, kind="ExternalInput")
    d_rowmask = nc.dram_tensor("rowmask", [SPC, 1, 2 * HALO * W], F32, kind="ExternalInput")
    d_wqkvT = nc.dram_tensor("wqkvT", [2, 128, 768], BF16, kind="ExternalInput")
    d_wvT = nc.dram_tensor("wvT", [2, 128, 264], BF16, kind="ExternalInput")
    d_wpT = nc.dram_tensor("wpT", [2, 128, 256], BF16, kind="ExternalInput")
    d_dwdiag = nc.dram_tensor("dwdiag", [128, max(1, N_PE_TAPS * 2 * 128)], BF16, kind="ExternalInput")
    nd = max(1, len(DVE_TAPS))
    d_dwvec = nc.dram_tensor("dwvec", [2, 128, nd], F32, kind="ExternalInput")
    d_bias6 = nc.dram_tensor("bias6", [128, 6], F32, kind="ExternalInput")
    d_brow = nc.dram_tensor("brow", [1, 512], F32, kind="ExternalInput")
    d_ones = nc.dram_tensor("ones", [1, 1024], F32, kind="ExternalInput")
    d_onehot = nc.dram_tensor("onehot", [1, 264], F32, kind="ExternalInput")
    d_selk = nc.dram_tensor("selk", [128, 2 * 97], F32, kind="ExternalInput")
    d_out = nc.dram_tensor("out", [SPC * 2, 128, NA], F32, kind="ExternalOutput")
    if DEBUG:
        d_dbg_qk = nc.dram_tensor("dbg_qk", [4, 128, NA], F32, kind="ExternalOutput")
        d_dbg_vp = nc.dram_tensor("dbg_vp", [2, 128, PFREE], F32, kind="ExternalOutput")
        d_dbg_vt = nc.dram_tensor("dbg_vt", [8, 128, 264], F32, kind="ExternalOutput")
        d_dbg_e = nc.dram_tensor("dbg_e", [2, 128, NA], F32, kind="ExternalOutput")
        d_dbg_pv = nc.dram_tensor("dbg_pv", [1, 128, NA], F32, kind="ExternalOutput")
        d_dbg_y = nc.dram_tensor("dbg_y", [2, 128, NA], F32, kind="ExternalOutput")
        d_dbg_acc = nc.dram_tensor("dbg_acc", [2, 128, NA], F32, kind="ExternalOutput")
        d_dbg_x = nc.dram_tensor("dbg_x", [2, 128, XFREE], F32, kind="ExternalOutput")
        d_dbg_w = nc.dram_tensor("dbg_w", [2, 128, 768], F32, kind="ExternalOutput")

    from contextlib import ExitStack
    with tile.TileContext(nc) as tc, ExitStack() as ctx:
        wp = ctx.enter_context(tc.tile_pool(name="weights", bufs=1))
        xp = ctx.enter_context(tc.tile_pool(name="x", bufs=2))
        qkp = ctx.enter_context(tc.tile_pool(name="qk", bufs=2))
        vpadp = ctx.enter_context(tc.tile_pool(name="vpad", bufs=2))
        vtp = ctx.enter_context(tc.tile_pool(name="vt", bufs=2))
        ep = ctx.enter_context(tc.tile_pool(name="e", bufs=8))
        zp = ctx.enter_context(tc.tile_pool(name="z", bufs=2))
        bcp = ctx.enter_context(tc.tile_pool(name="bc", bufs=2))
        yp = ctx.enter_context(tc.tile_pool(name="y", bufs=2))
        accp = ctx.enter_context(tc.tile_pool(name="acc", bufs=2))
        rmp = ctx.enter_context(tc.tile_pool(name="rm", bufs=2))
        outp = ctx.enter_context(tc.tile_pool(name="outp", bufs=4))
        ps_s = ctx.enter_context(tc.tile_pool(name="ps_s", bufs=2, space="PSUM"))
        ps_pv = ctx.enter_context(tc.tile_pool(name="ps_pv", bufs=1, space="PSUM"))
        ps_m = ctx.enter_context(tc.tile_pool(name="ps_m", bufs=2, space="PSUM"))

        # ---- load weights/constants (once) ----
        w_qkvT = [wp.tile([128, 768], BF16, tag=f"wqkvT{i}", name=f"wqkvT{i}") for i in range(2)]
        w_vT = [wp.tile([128, 264], BF16, tag=f"wvT{i}", name=f"wvT{i}") for i in range(2)]
        w_pT = [wp.tile([128, 256], BF16, tag=f"wpT{i}", name=f"wpT{i}") for i in range(2)]
        w_dwvec = [wp.tile([128, nd], F32, tag=f"dwvec{i}", name=f"dwvec{i}") for i in range(2)]
        for i in range(2):
            nc.sync.dma_start(w_qkvT[i][:], d_wqkvT.ap()[i])
            nc.sync.dma_start(w_vT[i][:], d_wvT.ap()[i])
            nc.sync.dma_start(w_pT[i][:], d_wpT.ap()[i])
            nc.sync.dma_start(w_dwvec[i][:], d_dwvec.ap()[i])
        w_dwdiag = wp.tile([128, max(1, N_PE_TAPS * 2 * 128)], BF16, tag="dwdiag", name="dwdiag")
        nc.sync.dma_start(w_dwdiag[:], d_dwdiag.ap())
        w_bias6 = wp.tile([128, 6], F32, tag="bias6", name="bias6")
        nc.sync.dma_start(w_bias6[:], d_bias6.ap())
        w_brow = wp.tile([1, 512], F32, tag="brow", name="brow")
        nc.sync.dma_start(w_brow[:], d_brow.ap())
        w_ones = wp.tile([1, 1024], F32, tag="ones", name="ones")
        nc.sync.dma_start(w_ones[:], d_ones.ap())
        w_onehot = wp.tile([1, 264], F32, tag="onehot", name="onehot")
        nc.sync.dma_start(w_onehot[:], d_onehot.ap())
        w_selk = wp.tile([128, 2 * 97], F32, tag="selk", name="selk")
        nc.sync.dma_start(w_selk[:], d_selk.ap())

        def diag_ap(i, ch):
            o = (2 * i + ch) * 128
            return w_dwdiag[:, o:o + 128]

        for s in range(SPC):
            xt = [xp.tile([128, XFREE], BF16, tag=f"x{c}", name=f"x{c}") for c in range(2)]
            for c in range(2):
                nc.sync.dma_start(xt[c][:], d_x2.ap()[s, c])
            rm = rmp.tile([1, 2 * HALO * W], F32, tag="rm", name="rm")
            nc.sync.dma_start(rm[:], d_rowmask.ap()[s])

            # ---- qkv conv: q,k -> plain tiles; v -> zero-padded tiles ----
            qk = [qkp.tile([128, NA], BF16, tag=f"qk{j}", name=f"qk{j}") for j in range(4)]
            vpad = [vpadp.tile([128, PFREE], BF16, tag=f"vp{c}", name=f"vp{c}") for c in range(2)]
            vpad1 = [vpadp.tile([128, PFREE], BF16, tag=f"vp1{c}", name=f"vp1{c}") for c in range(2)]
            for c in range(2):
                nc.gpsimd.memset(vpad[c][:], 0.0)
            for j in range(6):
                for hf in range(2):
                    pt = ps_m.tile([128, 512], F32, tag="mm", name="mm")
                    for kc in range(2):
                        nc.tensor.matmul(
                            pt[:], w_qkvT[kc][:, 128 * j:128 * j + 128],
                            xt[kc][:, XI0 + 512 * hf:XI0 + 512 * hf + 512],
                            start=(kc == 0), stop=(kc == 1))
                    if j < 4:
                        nc.vector.tensor_scalar_add(
                            qk[j][:, 512 * hf:512 * hf + 512], pt[:],
                            w_bias6[:, j:j + 1])
                    else:
                        c = j - 4
                        dst = vpad[c][:].rearrange("p (r w) -> p r w", w=PCOLS)[
                            :, HALO + 8 * hf:HALO + 8 * hf + 8, HALO:HALO + W]
                        nc.vector.tensor_scalar_add(
                            dst, pt[:].rearrange("p (r w) -> p r w", w=W),
                            w_bias6[:, j:j + 1])

            # ---- halo v rows (top 3 / bottom 3), masked by rowmask ----
            for c in range(2):
                for side in range(2):
                    pt = ps_m.tile([128, HALO * W], F32, tag="mm", name="mm")
                    nc.tensor.matmul(
                        pt[:], w_brow[:, 128 * c:128 * c + 128],
                        rm[:, side * HALO * W:(side + 1) * HALO * W],
                        start=True, stop=False)
                    xoff = 0 if side == 0 else (PROWS - HALO) * W
                    for kc in range(2):
                        nc.tensor.matmul(
                            pt[:],
                            w_qkvT[kc][:, 512 + 128 * c:512 + 128 * c + 128],
                            xt[kc][:, xoff:xoff + HALO * W],
                            start=False, stop=(kc == 1))
                    roff = 0 if side == 0 else PROWS - HALO
                    dst = vpad[c][:].rearrange("p (r w) -> p r w", w=PCOLS)[
                        :, roff:roff + HALO, HALO:HALO + W]
                    nc.vector.tensor_copy(
                        dst, pt[:].rearrange("p (r w) -> p r w", w=W))
            for c in range(2):
                nc.gpsimd.tensor_copy(vpad1[c][:, 0:PFREE - 1], vpad[c][:, 1:PFREE])

            # ---- vT tiles (pix-block major) with ones column ----
            vt = [vtp.tile([128, 264], BF16, tag=f"vt{jb}", name=f"vt{jb}") for jb in range(8)]
            for jb in range(8):
                pt = ps_m.tile([128, 264], F32, tag="mm", name="mm")
                nc.tensor.matmul(pt[:], w_ones[:, :128], w_onehot[:],
                                 start=True, stop=False)
                for kc in range(2):
                    nc.tensor.matmul(
                        pt[:], xt[kc][:, XI0 + 128 * jb:XI0 + 128 * jb + 128],
                        w_vT[kc][:], start=False, stop=(kc == 1))
                nc.vector.tensor_copy(vt[jb][:], pt[:])

            if DEBUG and s == 0:
                dbp = ctx.enter_context(tc.tile_pool(name=f"dbg{s}", bufs=2))
                def dump(dst, src_ap, nm):
                    t = dbp.tile([128, PFREE], F32, tag="dbg", name=nm)
                    t = t[:src_ap.shape[0], :src_ap.shape[-1]]
                    nc.vector.tensor_copy(t[:], src_ap)
                    nc.sync.dma_start(dst, t[:])
                for j4 in range(4):
                    dump(d_dbg_qk.ap()[j4], qk[j4][:], f"dqk{j4}")
                for c in range(2):
                    dump(d_dbg_x.ap()[c], xt[c][:], f"dx{c}")
                    dump(d_dbg_w.ap()[c], w_qkvT[c][:], f"dw{c}")
                for c in range(2):
                    dump(d_dbg_vp.ap()[c], vpad[c][:], f"dvp{c}")
                for jb in range(8):
                    dump(d_dbg_vt.ap()[jb], vt[jb][:], f"dvt{jb}")

            # ---- attention pairs + interleaved depthwise ----
            rza = zp.tile([128, NA], F32, tag="rza", name="rza")
            rzb = zp.tile([128, NA], F32, tag="rzb", name="rzb")

            y = [yp.tile([128, NA], BF16, tag=f"y{c}", name=f"y{c}") for c in range(2)]
            acc = [accp.tile([128, NA], BF16, tag=f"acc{c}", name=f"acc{c}") for c in range(2)]

            def vwin(c, dy, dx, hf=None, rows=ROWS):
                """(128, rows, W) window of padded v at tap (dy,dx)."""
                if dx % 2 == 0:
                    srct, dxx = vpad[c], dx
                else:
                    srct, dxx = vpad1[c], dx - 1
                r0 = dy + (0 if hf is None else 8 * hf)
                return srct[:].rearrange("p (r w) -> p r w", w=PCOLS)[
                    :, r0:r0 + rows, dxx:dxx + W]

            dve_jobs = [(c, i, t) for c in range(2) for i, t in enumerate(DVE_TAPS)]
            pe_jobs = [(c, hf, i, t) for c in range(2) for hf in range(2)
                       for i, t in enumerate(PE_TAPS)]
            dwst = {}

            # init accumulators with first DVE tap of each chunk (mult only)
            for c in range(2):
                idx = next(k for k, jb in enumerate(dve_jobs) if jb[0] == c)
                _, i, t = dve_jobs.pop(idx)
                nc.vector.tensor_scalar_mul(
                    acc[c][:].rearrange("p (r w) -> p r w", w=W),
                    vwin(c, t // 7, t % 7), w_dwvec[c][:, i:i + 1])

            def emit_pe_dw(njobs):
                for _ in range(njobs):
                    if not pe_jobs:
                        return
                    c, hf, i, t = pe_jobs.pop(0)
                    key = (c, hf)
                    st = dwst.get(key)
                    if st is None:
                        st = dwst[key] = [ps_m.tile([128, 512], F32, tag="mm", name="mm"), 0]
                    more = any(j[0] == c and j[1] == hf for j in pe_jobs)
                    is_last = (st[1] == PE_SEG - 1) or not more
                    nc.tensor.matmul(
                        st[0][:], diag_ap(i, c),
                        vwin(c, t // 7, t % 7, hf=hf, rows=8),
                        start=(st[1] == 0), stop=is_last)
                    st[1] += 1
                    if is_last:
                        nc.vector.tensor_tensor(
                            acc[c][:, 512 * hf:512 * hf + 512],
                            st[0][:].rearrange("p (r w) -> p (r w)") if False
                            else st[0][:],
                            acc[c][:, 512 * hf:512 * hf + 512],
                            mybir.AluOpType.add)
                        dwst[key] = None
                        del dwst[key]

            def emit_dve_dw(njobs):
                for _ in range(njobs):
                    if not dve_jobs:
                        return
                    c, i, t = dve_jobs.pop(0)
                    a3 = acc[c][:].rearrange("p (r w) -> p r w", w=W)
                    nc.vector.scalar_tensor_tensor(
                        a3, vwin(c, t // 7, t % 7), w_dwvec[c][:, i:i + 1], a3,
                        mybir.AluOpType.mult, mybir.AluOpType.add)

            n_dve = max(1, (len(dve_jobs) + 31) // 32)
            n_pe = max(1, (len(pe_jobs) + 31) // 32)

            for p in range(4):
                h0 = 2 * p
                pv = ps_pv.tile([128, NA], F32, tag="pv", name="pv")
                for j in range(8):
                    spt = [ps_s.tile([128, NA], F32, tag="s", name="s") for _ in range(2)]
                    et = [ep.tile([128, NA], BF16, tag="e", name="e") for _ in range(2)]
                    for hh in range(2):
                        h = h0 + hh
                        kb, ko = h // 4, 32 * (h % 4)
                        for hf in range(2):
                            nc.tensor.matmul(
                                spt[hh][:, 512 * hf:512 * hf + 512],
                                qk[2 + kb][ko:ko + 32, 128 * j:128 * j + 128],
                                qk[kb][ko:ko + 32, 512 * hf:512 * hf + 512],
                                start=True, stop=True, tile_position=(ko, 0))
                        nc.scalar.activation(
                            et[hh][:], spt[hh][:],
                            mybir.ActivationFunctionType.Exp, scale=SCALE)
                        if DEBUG and s == 0 and p == 0 and j == 0:
                            dump(d_dbg_e.ap()[hh], et[hh][:], f"de{hh}")
                    for hh in range(2):
                        h = h0 + hh
                        for hf in range(2):
                            nc.tensor.matmul(
                                pv[64 * hh:64 * hh + 33, 512 * hf:512 * hf + 512],
                                vt[j][:, 33 * h:33 * h + 33],
                                et[hh][:, 512 * hf:512 * hf + 512],
                                start=(j == 0), stop=(j == 7),
                                tile_position=(0, 64 * hh))
                    if DW_INTERLEAVE:
                        emit_pe_dw(n_pe)
                        emit_dve_dw(n_dve)

                if DEBUG and s == 0 and p == 0:
                    dump(d_dbg_pv.ap()[0], pv[:], "dpv")
                nc.vector.reciprocal(rza[32 * p:32 * p + 1, :], pv[32:33, :])
                nc.vector.reciprocal(rzb[32 * p:32 * p + 1, :], pv[96:97, :])
                bcs = bcp.tile([128, NA], F32, tag="bcs", name="bcs")
                yb, yo = p // 2, 64 * (p % 2)
                for hf in range(2):
                    bpt = ps_m.tile([128, 512], F32, tag="mm", name="mm")
                    nc.tensor.matmul(
                        bpt[0:97, :], w_selk[32 * p:32 * p + 1, 0:97],
                        rza[32 * p:32 * p + 1, 512 * hf:512 * hf + 512],
                        start=True, stop=False, tile_position=(32 * p, 0))
                    nc.tensor.matmul(
                        bpt[0:97, :], w_selk[32 * p:32 * p + 1, 97:194],
                        rzb[32 * p:32 * p + 1, 512 * hf:512 * hf + 512],
                        start=False, stop=True, tile_position=(32 * p, 0))
                    nc.vector.tensor_copy(
                        bcs[0:97, 512 * hf:512 * hf + 512], bpt[0:97, :])
                    nc.vector.tensor_tensor(
                        y[yb][yo:yo + 32, 512 * hf:512 * hf + 512],
                        pv[0:32, 512 * hf:512 * hf + 512],
                        bcs[0:32, 512 * hf:512 * hf + 512],
                        mybir.AluOpType.mult)
                    nc.vector.tensor_tensor(
                        y[yb][yo + 32:yo + 64, 512 * hf:512 * hf + 512],
                        pv[64:96, 512 * hf:512 * hf + 512],
                        bcs[64:96, 512 * hf:512 * hf + 512],
                        mybir.AluOpType.mult)

            emit_pe_dw(10 ** 9)
            emit_dve_dw(10 ** 9)

            # ---- y += depthwise ; proj ; out ----
            for c in range(2):
                nc.vector.tensor_tensor(y[c][:], y[c][:], acc[c][:],
                                        mybir.AluOpType.add)
            if DEBUG and s == 0:
                for c in range(2):
                    dump(d_dbg_y.ap()[c], y[c][:], f"dy{c}")
                    dump(d_dbg_acc.ap()[c], acc[c][:], f"dacc{c}")
            for ob in range(2):
                for hf in range(2):
                    pt = ps_m.tile([128, 512], F32, tag="mm", name="mm")
                    nc.tensor.matmul(
                        pt[:], w_brow[:, 256 + 128 * ob:256 + 128 * ob + 128],
                        w_ones[:, :512], start=True, stop=False)
                    for kc in range(2):
                        nc.tensor.matmul(
                            pt[:], w_pT[kc][:, 128 * ob:128 * ob + 128],
                            y[kc][:, 512 * hf:512 * hf + 512],
                            start=False, stop=(kc == 1))
                    ot = outp.tile([128, 512], F32, tag="ot", name="ot")
                    nc.vector.tensor_copy(ot[:], pt[:])
                    nc.sync.dma_start(
                        d_out.ap()[2 * s + ob, :, 512 * hf:512 * hf + 512], ot[:])
    nc.compile()
    return nc


def kernel(**inputs) -> np.ndarray:
    if "nc" not in _CACHE:
        _CACHE["nc"] = _build()
    nc = _CACHE["nc"]
    in_maps = _host_prep(inputs)
    res = run_bass_kernel_spmd(nc, in_maps, core_ids=list(range(NCORES)))
    out = np.empty((B, C, H, W), np.float32)
    for core in range(NCORES):
        o = res.results[core]["out"].reshape(C, ROWS, W) if False else \
            res.results[core]["out"]
        for s in range(SPC):
            gs = core * SPC + s
            b_, a_ = gs // AREA, gs % AREA
            out[b_, :, a_ * ROWS:(a_ + 1) * ROWS] = np.concatenate(
                [o[2 * s], o[2 * s + 1]], axis=0).reshape(C, ROWS, W)
    return out

